# revision 26
# baseline (speedup 1.0000x reference)
"""Bass/Tile TRN2 kernel for per-token multi-head attention over heads.

Reference computation (per token t):
  qkv = x @ w_qkv + b_qkv                  # [t, 3072]
  q/k/v[h, d] = qkv[h*192 + {0,64,128} + d]
  scores[h, g] = q[h] . k[g] / 8
  attn = softmax(scores, axis=g)
  out[h, d] = sum_g attn[h, g] v[g, d]
  y = out.reshape(1024) @ w_out + b_out

Sharding: tokens (B*S = 32768) split evenly over 8 cores; weights replicated.

Layout notes (v2):
  - qkv computed transposed ([f x t]) so per-head 64-row slabs are clean
    partition ranges; f32r matmuls (full PE rate at N>=256, ~1e-4 rounding).
  - per-head q/k/v repacked h-major into [d, head, t] bf16 tiles with
    contiguous copies; block-diag 128x128 matmuls give 8 tokens' 16x16
    head-attention at once (2 groups share each psum tile). Mask selects
    t==t' pairs; exp is unnormalized, softmax denominator comes from a
    ones-column in the AV matmul.
  - x transposed via DMA-transpose (2 x 64-partition transfers per e-chunk).
"""

import numpy as np
import ml_dtypes

H, DH = 16, 64
E = 1024
F3 = 3072
B, S = 4, 8192
N_CORES = 8
TOKS = (B * S) // N_CORES  # 4096 tokens per core
T = 256                    # tokens per unrolled iteration
NG = T // 8                # 8-token groups per iteration

NEG = -1.0e9


def _consts():
    # scoresT rows a=(slot_k, t), cols b=(slot_q, t'); valid iff t==t'
    a = np.arange(128)
    mask = np.where((a[:, None] % 8) == (a[None, :] % 8), 0.0, NEG).astype(
        np.float32
    )
    mask2 = np.concatenate([mask, mask], axis=1)  # [128, 256] for group pairs
    ident = np.eye(128, dtype=np.float32)
    return mask2, ident


def build(toks_per_core=TOKS):
    from concourse.bacc import Bacc
    import concourse.mybir as mybir
    from concourse.tile import TileContext
    from concourse.bass import ds

    f32 = mybir.dt.float32
    bf16 = mybir.dt.bfloat16
    niter = toks_per_core // T
    NQ = NG // 4  # quads per iteration

    nc = Bacc("TRN2")
    x_d = nc.dram_tensor("x", [toks_per_core, E], bf16, kind="ExternalInput")
    wqkv_d = nc.dram_tensor("w_qkv", [E, F3], bf16, kind="ExternalInput")
    bqkv_d = nc.dram_tensor("b_qkv", [128, F3 // 128], f32, kind="ExternalInput")
    wout_d = nc.dram_tensor("w_out", [E, E], bf16, kind="ExternalInput")
    bout_d = nc.dram_tensor("b_out", [1, E], bf16, kind="ExternalInput")
    out_d = nc.dram_tensor("out", [toks_per_core, E], f32, kind="ExternalOutput")

    _, ident_np = _consts()
    # mask fused into the scores matmul: extra contraction rows 64..71.
    # km[r, thi, s, tlo] = -sqrt(1e9) if tlo == r else 0
    # qm[r, s, t]        = +sqrt(1e9) if t % 8 != r else 0
    # => sum_r km.qm = -1e9 on t != t' pairs, exactly 0 on valid pairs
    SQ = np.float32(31623.0)
    tlo = np.arange(8)
    km_np = np.zeros((8, T // 8, H, 8), np.float32)
    km_np[tlo, :, :, tlo] = -SQ
    t_arr = np.arange(T)
    qm_np = np.where(
        (t_arr[None, None, :] % 8) != tlo[:, None, None], SQ, 0.0
    ).astype(np.float32)
    qm_np = np.broadcast_to(qm_np, (8, H, T)).copy()
    km_c = nc.inline_tensor(
        km_np.reshape(8, -1).astype(ml_dtypes.bfloat16), name="km_c"
    )
    qm_c = nc.inline_tensor(
        qm_np.reshape(8, -1).astype(ml_dtypes.bfloat16), name="qm_c"
    )
    identb_c = nc.inline_tensor(ident_np.astype(ml_dtypes.bfloat16), name="identb_c")
    ones_c = nc.inline_tensor(np.ones((1, 128), ml_dtypes.bfloat16), name="ones_c")

    # parity-major head slot: even heads 0-7, odd heads 8-15
    def hslot(h):
        return (h % 2) * 8 + h // 2

    with TileContext(nc) as tc:
        with (
            tc.tile_pool(name="persist", bufs=1) as pp,
            tc.tile_pool(name="xp", bufs=2) as xp,
            tc.tile_pool(name="xtp", bufs=2) as xtp,
            tc.tile_pool(name="attnsb", bufs=4) as attnsb,
            tc.tile_pool(name="outtokp", bufs=2) as outtokp,
            tc.tile_pool(name="outfp", bufs=3) as outfp,
            tc.tile_pool(name="psbig", bufs=3, space="PSUM") as psbig,
            tc.tile_pool(name="psattn", bufs=5, space="PSUM") as psattn,
        ):
            # ---- resident weights / constants ----
            w_sb = pp.tile([128, 8, F3], bf16)
            nc.sync.dma_start(w_sb, wqkv_d.rearrange("(ko kp) f -> kp ko f", kp=128))
            wout_sb = pp.tile([128, 8, E], bf16)
            nc.sync.dma_start(wout_sb, wout_d.rearrange("(ko kp) f -> kp ko f", kp=128))
            bqkv_sb = pp.tile([128, F3 // 128], f32)
            nc.sync.dma_start(bqkv_sb, bqkv_d[:])
            bout_sb = pp.tile([1, E], bf16)
            nc.sync.dma_start(bout_sb, bout_d[:])
            idb_sb = pp.tile([128, 128], bf16)
            nc.sync.dma_start(idb_sb, identb_c[:])
            ones_sb = pp.tile([1, 128], bf16)
            nc.sync.dma_start(ones_sb, ones_c[:])

            # double-buffered packs; mask rows are persistent constants
            qpacks, kpacks, vpacks = [], [], []
            for sidx in range(2):
                qpack = pp.tile([72, H, T], bf16, name=f"qpack{sidx}")
                kpack = pp.tile([72, T // 8, H, 8], bf16, name=f"kpack{sidx}")
                vpack = pp.tile([65, T // 8, H, 8], bf16, name=f"vpack{sidx}")
                nc.gpsimd.memset(vpack[64:65, :, :, :], 1.0)
                nc.sync.dma_start(
                    qpack[64:72, :, :].rearrange("p a b -> p (a b)"), qm_c[:]
                )
                nc.sync.dma_start(
                    kpack[64:72, :, :, :].rearrange("p a b c -> p (a b c)"),
                    km_c[:],
                )
                qpacks.append(qpack)
                kpacks.append(kpack)
                vpacks.append(vpack)

            def emit_xt(it, xt_sb, e):
                t0 = it * T
                nc.sync.dma_start_transpose(
                    xt_sb[:, e, :],
                    x_d[ds(t0, T), ds(e * 128, 128)],
                )

            drain_alt = [0]

            def emit_qkv_ftile(it, xt_sb, j):
                # f-tile j -> psum -> two 64-row slab drains into packs
                sidx = it % 2
                psq_full = psbig.tile([128, 512], f32, tag="ps_big")
                psq = psq_full[:, :T]
                for e in range(8):
                    nc.tensor.matmul(
                        psq,
                        w_sb[:, e, ds(j * 128, 128)],
                        xt_sb[:, e, :],
                        start=(e == 0),
                        stop=(e == 7),
                    )
                for half in range(2):
                    frow = j * 128 + half * 64
                    h, rem = divmod(frow, 192)
                    which = rem // 64
                    sl = hslot(h)
                    if which == 0:
                        dst = qpacks[sidx][:64, sl, :]
                    elif which == 1:
                        dst = kpacks[sidx][:64, :, sl, :]
                    else:
                        dst = vpacks[sidx][:64, :, sl, :]
                    src = psq[half * 64 : half * 64 + 64, :]
                    bias_ap = bqkv_sb[half * 64 : half * 64 + 64, j : j + 1]
                    if which == 0:
                        nc.scalar.activation(
                            dst,
                            src,
                            mybir.ActivationFunctionType.Identity,
                            bias=bias_ap,
                            scale=1.0,
                        )
                    else:
                        nc.vector.scalar_tensor_tensor(
                            out=dst,
                            in0=src.rearrange("p (a b) -> p a b", b=8),
                            scalar=1.0,
                            in1=bias_ap[:, :, None].to_broadcast(
                                (64, T // 8, 8)
                            ),
                            op0=mybir.AluOpType.mult,
                            op1=mybir.AluOpType.add,
                        )

            def emit_attn_quad(it, outtok, q4):
                sidx = it % 2
                qpack, kpack, vpack = qpacks[sidx], kpacks[sidx], vpacks[sidx]
                gs4 = [4 * q4 + i for i in range(4)]
                psS4 = psattn.tile([128, 4, 128], f32, tag="ps_attn")
                for i, g in enumerate(gs4):
                    nc.tensor.matmul(
                        psS4[:, i, :],
                        kpack[:, g, :, :].rearrange("p a b -> p (a b)"),
                        qpack[:, :, ds(g * 8, 8)],
                        start=True,
                        stop=True,
                    )
                expS4 = attnsb.tile([128, 512], bf16, tag="expS")
                for eh in range(2):
                    nc.scalar.activation(
                        expS4[:, ds(eh * 256, 256)],
                        psS4[:, ds(eh * 2, 2), :].rearrange("p a b -> p (a b)"),
                        mybir.ActivationFunctionType.Exp,
                        bias=0.0,
                        scale=0.125,
                    )
                psV4 = psattn.tile([128, 4, 66], bf16, tag="ps_attn")
                for i, g in enumerate(gs4):
                    nc.tensor.transpose(
                        psV4[:, i, :65],
                        vpack[:, g, :, :].rearrange("p a b -> p (a b)"),
                        idb_sb[:65, :65],
                    )
                vt4_sb = attnsb.tile([128, 4, 65], bf16, tag="vt")
                nc.scalar.activation(
                    vt4_sb[:],
                    psV4[:, :, :65],
                    mybir.ActivationFunctionType.Copy,
                )
                psAV4 = psattn.tile([128, 4, 65], f32, tag="ps_attn")
                for i in range(4):
                    nc.tensor.matmul(
                        psAV4[:, i, :],
                        expS4[:, ds(i * 128, 128)],
                        vt4_sb[:, i, :],
                        start=True,
                        stop=True,
                    )
                rec4 = attnsb.tile([128, 4], f32, tag="rec")
                nc.vector.reciprocal(rec4[:], psAV4[:, :, 64])
                onorm4 = attnsb.tile([128, 4, 64], bf16, tag="onorm")
                nc.vector.tensor_tensor(
                    onorm4[:],
                    psAV4[:, :, 0:64],
                    rec4[:, :, None].to_broadcast((128, 4, 64)),
                    mybir.AluOpType.mult,
                )
                psN4 = psattn.tile([128, 2, 128], bf16, tag="ps_attn")
                for p in range(2):
                    nc.tensor.transpose(
                        psN4[:, p, :],
                        onorm4[:, 2 * p : 2 * p + 2, :].rearrange(
                            "p a b -> p (a b)"
                        ),
                        idb_sb[:],
                    )
                cp_alt = 0
                for p in range(2):
                    for i in range(2):
                        g = gs4[2 * p + i]
                        for par in range(2):
                            src = psN4[
                                i * 64 : i * 64 + 64, p, ds(par * 64, 64)
                            ].rearrange("p (a b) -> p a b", a=8)
                            dst = outtok[
                                par * 64 : par * 64 + 64, :, ds(g * 8, 8)
                            ]
                            if cp_alt % 2 == 0:
                                nc.vector.tensor_copy(out=dst, in_=src)
                            else:
                                nc.scalar.copy(out=dst, in_=src)
                            cp_alt += 1

            def emit_proj(it, outtok, jm):
                t0 = it * T
                for nh in range(2):
                    psO = psbig.tile([128, 512], f32, tag="ps_big")
                    for k2 in range(8):
                        nc.tensor.matmul(
                            psO,
                            outtok[:, k2, ds(jm * 128, 128)],
                            wout_sb[:, k2, ds(nh * 512, 512)],
                            start=(k2 == 0),
                            stop=False,
                        )
                    nc.tensor.matmul(
                        psO,
                        ones_sb[:, :],
                        bout_sb[:, ds(nh * 512, 512)],
                        start=False,
                        stop=True,
                    )
                    outf = outfp.tile([128, 512], f32, tag="outf")
                    nc.scalar.activation(
                        outf[:], psO, mybir.ActivationFunctionType.Copy
                    )
                    nc.sync.dma_start(
                        out_d[ds(t0 + jm * 128, 128), ds(nh * 512, 512)],
                        outf[:],
                    )

            # ---- software-pipelined schedule ----
            # prologue: QKV for iter 0
            xt_sb = xtp.tile([128, 8, T], bf16)
            for e in range(8):
                emit_xt(0, xt_sb, e)
            for j in range(F3 // 128):
                emit_qkv_ftile(0, xt_sb, j)

            for it in range(niter):
                nxt = it + 1
                outtok = outtokp.tile([128, 8, T], bf16, tag="outtok")
                if nxt < niter:
                    xt_nxt = xtp.tile([128, 8, T], bf16)
                for q4 in range(NQ):
                    emit_attn_quad(it, outtok, q4)
                    if nxt < niter:
                        if q4 < 2:
                            for e in range(4 * q4, 4 * q4 + 4):
                                emit_xt(nxt, xt_nxt, e)
                        else:
                            for j in range(4 * (q4 - 2), 4 * (q4 - 2) + 4):
                                emit_qkv_ftile(nxt, xt_nxt, j)
                    if q4 == NQ // 2 - 1:
                        emit_proj(it, outtok, 0)
                emit_proj(it, outtok, 1)
    nc.finalize()
    return nc


_cache = {}


def _get_nc(toks_per_core=TOKS):
    if toks_per_core not in _cache:
        _cache[toks_per_core] = build(toks_per_core)
    return _cache[toks_per_core]


def prep_inputs(x, w_qkv, b_qkv, w_out, b_out, toks_per_core=TOKS, n_cores=N_CORES):
    """Shard tokens over cores; replicate (host-preprocessed) weights."""
    xf = np.ascontiguousarray(np.asarray(x, dtype=np.float32).astype(ml_dtypes.bfloat16)).reshape(-1, E)
    wq = np.ascontiguousarray(np.asarray(w_qkv, dtype=np.float32).astype(ml_dtypes.bfloat16))
    bq = np.ascontiguousarray(
        np.asarray(b_qkv, dtype=np.float32).reshape(F3 // 128, 128).T
    )
    wo = np.ascontiguousarray(np.asarray(w_out).astype(ml_dtypes.bfloat16))
    bo = np.ascontiguousarray(np.asarray(b_out, dtype=np.float32).astype(ml_dtypes.bfloat16).reshape(1, E))
    in_maps = []
    for c in range(n_cores):
        in_maps.append(
            {
                "x": np.ascontiguousarray(
                    xf[c * toks_per_core : (c + 1) * toks_per_core]
                ),
                "w_qkv": wq,
                "b_qkv": bq,
                "w_out": wo,
                "b_out": bo,
            }
        )
    return in_maps


def run(x, w_qkv, b_qkv, w_out, b_out, toks_per_core=TOKS, n_cores=N_CORES, **kw):
    from concourse import bass_utils

    nc = _get_nc(toks_per_core)
    in_maps = prep_inputs(
        x, w_qkv, b_qkv, w_out, b_out, toks_per_core, n_cores
    )
    res = bass_utils.run_bass_kernel_spmd(
        nc, in_maps, core_ids=list(range(n_cores)), **kw
    )
    out = np.concatenate([r["out"] for r in res.results], axis=0)
    return out, res


def kernel(x, w_qkv, b_qkv, w_out, b_out):
    out, _ = run(x, w_qkv, b_qkv, w_out, b_out)
    return out.reshape(x.shape[0], x.shape[1], E)


# revision 27
# speedup vs baseline: 1.0637x; 1.0637x over previous
"""Bass/Tile TRN2 kernel for per-token multi-head attention over heads.

Reference computation (per token t):
  qkv = x @ w_qkv + b_qkv                  # [t, 3072]
  q/k/v[h, d] = qkv[h*192 + {0,64,128} + d]
  scores[h, g] = q[h] . k[g] / 8
  attn = softmax(scores, axis=g)
  out[h, d] = sum_g attn[h, g] v[g, d]
  y = out.reshape(1024) @ w_out + b_out

Sharding: tokens (B*S = 32768) split evenly over 8 cores; weights replicated.

Layout notes (v2):
  - qkv computed transposed ([f x t]) so per-head 64-row slabs are clean
    partition ranges; f32r matmuls (full PE rate at N>=256, ~1e-4 rounding).
  - per-head q/k/v repacked h-major into [d, head, t] bf16 tiles with
    contiguous copies; block-diag 128x128 matmuls give 8 tokens' 16x16
    head-attention at once (2 groups share each psum tile). Mask selects
    t==t' pairs; exp is unnormalized, softmax denominator comes from a
    ones-column in the AV matmul.
  - x transposed via DMA-transpose (2 x 64-partition transfers per e-chunk).
"""

import numpy as np
import ml_dtypes

H, DH = 16, 64
E = 1024
F3 = 3072
B, S = 4, 8192
N_CORES = 8
TOKS = (B * S) // N_CORES  # 4096 tokens per core
T = 256                    # tokens per unrolled iteration
NG = T // 8                # 8-token groups per iteration

NEG = -1.0e9


def _consts():
    # scoresT rows a=(slot_k, t), cols b=(slot_q, t'); valid iff t==t'
    a = np.arange(128)
    mask = np.where((a[:, None] % 8) == (a[None, :] % 8), 0.0, NEG).astype(
        np.float32
    )
    mask2 = np.concatenate([mask, mask], axis=1)  # [128, 256] for group pairs
    ident = np.eye(128, dtype=np.float32)
    return mask2, ident


def build(toks_per_core=TOKS):
    from concourse.bacc import Bacc
    import concourse.mybir as mybir
    from concourse.tile import TileContext
    from concourse.bass import ds

    f32 = mybir.dt.float32
    bf16 = mybir.dt.bfloat16
    niter = toks_per_core // T
    NQ = NG // 4  # quads per iteration

    nc = Bacc("TRN2")
    x_d = nc.dram_tensor("x", [toks_per_core, E], bf16, kind="ExternalInput")
    wqkv_d = nc.dram_tensor("w_qkv", [E, F3], bf16, kind="ExternalInput")
    bqkv_d = nc.dram_tensor("b_qkv", [128, F3 // 128], f32, kind="ExternalInput")
    wout_d = nc.dram_tensor("w_out", [E, E], bf16, kind="ExternalInput")
    bout_d = nc.dram_tensor("b_out", [1, E], bf16, kind="ExternalInput")
    out_d = nc.dram_tensor("out", [toks_per_core, E], f32, kind="ExternalOutput")

    _, ident_np = _consts()
    # mask fused into the scores matmul: extra contraction rows 64..71.
    # km[r, thi, s, tlo] = -sqrt(1e9) if tlo == r else 0
    # qm[r, s, t]        = +sqrt(1e9) if t % 8 != r else 0
    # => sum_r km.qm = -1e9 on t != t' pairs, exactly 0 on valid pairs
    SQ = np.float32(31623.0)
    tlo = np.arange(8)
    km_np = np.zeros((8, T // 8, H, 8), np.float32)
    km_np[tlo, :, :, tlo] = -SQ
    t_arr = np.arange(T)
    qm_np = np.where(
        (t_arr[None, None, :] % 8) != tlo[:, None, None], SQ, 0.0
    ).astype(np.float32)
    qm_np = np.broadcast_to(qm_np, (8, H, T)).copy()
    km_c = nc.inline_tensor(
        km_np.reshape(8, -1).astype(ml_dtypes.bfloat16), name="km_c"
    )
    qm_c = nc.inline_tensor(
        qm_np.reshape(8, -1).astype(ml_dtypes.bfloat16), name="qm_c"
    )
    identb_c = nc.inline_tensor(ident_np.astype(ml_dtypes.bfloat16), name="identb_c")
    ones_c = nc.inline_tensor(np.ones((1, 128), ml_dtypes.bfloat16), name="ones_c")

    # parity-major head slot: even heads 0-7, odd heads 8-15
    def hslot(h):
        return (h % 2) * 8 + h // 2

    with TileContext(nc) as tc:
        with (
            tc.tile_pool(name="persist", bufs=1) as pp,
            tc.tile_pool(name="xp", bufs=2) as xp,
            tc.tile_pool(name="xtp", bufs=2) as xtp,
            tc.tile_pool(name="attnsb", bufs=6) as attnsb,
            tc.tile_pool(name="outtokp", bufs=2) as outtokp,
            tc.tile_pool(name="outfp", bufs=3) as outfp,
            tc.tile_pool(name="psbig", bufs=3, space="PSUM") as psbig,
            tc.tile_pool(name="psattn", bufs=5, space="PSUM") as psattn,
        ):
            # ---- resident weights / constants ----
            w_sb = pp.tile([128, 8, F3], bf16)
            nc.sync.dma_start(w_sb, wqkv_d.rearrange("(ko kp) f -> kp ko f", kp=128))
            wout_sb = pp.tile([128, 8, E], bf16)
            nc.sync.dma_start(wout_sb, wout_d.rearrange("(ko kp) f -> kp ko f", kp=128))
            bqkv_sb = pp.tile([128, F3 // 128], f32)
            nc.sync.dma_start(bqkv_sb, bqkv_d[:])
            bout_sb = pp.tile([1, E], bf16)
            nc.sync.dma_start(bout_sb, bout_d[:])
            idb_sb = pp.tile([128, 128], bf16)
            nc.sync.dma_start(idb_sb, identb_c[:])
            ones_sb = pp.tile([1, 128], bf16)
            nc.sync.dma_start(ones_sb, ones_c[:])

            # double-buffered packs; mask rows are persistent constants
            qpacks, kpacks, vpacks = [], [], []
            for sidx in range(2):
                qpack = pp.tile([72, H, T], bf16, name=f"qpack{sidx}")
                kpack = pp.tile([72, T // 8, H, 8], bf16, name=f"kpack{sidx}")
                vpack = pp.tile([65, T // 8, H, 8], bf16, name=f"vpack{sidx}")
                nc.gpsimd.memset(vpack[64:65, :, :, :], 1.0)
                nc.sync.dma_start(
                    qpack[64:72, :, :].rearrange("p a b -> p (a b)"), qm_c[:]
                )
                nc.sync.dma_start(
                    kpack[64:72, :, :, :].rearrange("p a b c -> p (a b c)"),
                    km_c[:],
                )
                qpacks.append(qpack)
                kpacks.append(kpack)
                vpacks.append(vpack)

            def emit_xt(it, xt_sb, e):
                t0 = it * T
                nc.sync.dma_start_transpose(
                    xt_sb[:, e, :],
                    x_d[ds(t0, T), ds(e * 128, 128)],
                )

            drain_alt = [0]

            def emit_qkv_ftile(it, xt_sb, j):
                # f-tile j -> psum -> two 64-row slab drains into packs
                sidx = it % 2
                psq_full = psbig.tile([128, 512], f32, tag="ps_big")
                psq = psq_full[:, :T]
                for e in range(8):
                    nc.tensor.matmul(
                        psq,
                        w_sb[:, e, ds(j * 128, 128)],
                        xt_sb[:, e, :],
                        start=(e == 0),
                        stop=(e == 7),
                    )
                for half in range(2):
                    frow = j * 128 + half * 64
                    h, rem = divmod(frow, 192)
                    which = rem // 64
                    sl = hslot(h)
                    if which == 0:
                        dst = qpacks[sidx][:64, sl, :]
                    elif which == 1:
                        dst = kpacks[sidx][:64, :, sl, :]
                    else:
                        dst = vpacks[sidx][:64, :, sl, :]
                    src = psq[half * 64 : half * 64 + 64, :]
                    bias_ap = bqkv_sb[half * 64 : half * 64 + 64, j : j + 1]
                    if which == 0:
                        nc.scalar.activation(
                            dst,
                            src,
                            mybir.ActivationFunctionType.Identity,
                            bias=bias_ap,
                            scale=1.0,
                        )
                    else:
                        nc.vector.scalar_tensor_tensor(
                            out=dst,
                            in0=src.rearrange("p (a b) -> p a b", b=8),
                            scalar=1.0,
                            in1=bias_ap[:, :, None].to_broadcast(
                                (64, T // 8, 8)
                            ),
                            op0=mybir.AluOpType.mult,
                            op1=mybir.AluOpType.add,
                        )

            def emit_attn_quad(it, outtok, q4):
                sidx = it % 2
                qpack, kpack, vpack = qpacks[sidx], kpacks[sidx], vpacks[sidx]
                gs4 = [4 * q4 + i for i in range(4)]
                psS4 = psattn.tile([128, 4, 128], f32, tag="ps_attn")
                for i, g in enumerate(gs4):
                    nc.tensor.matmul(
                        psS4[:, i, :],
                        kpack[:, g, :, :].rearrange("p a b -> p (a b)"),
                        qpack[:, :, ds(g * 8, 8)],
                        start=True,
                        stop=True,
                    )
                expS4 = attnsb.tile([128, 512], bf16, tag="expS")
                nc.scalar.activation(
                    expS4[:],
                    psS4.rearrange("p a b -> p (a b)"),
                    mybir.ActivationFunctionType.Exp,
                    bias=0.0,
                    scale=0.125,
                )
                psV4 = psattn.tile([128, 4, 66], bf16, tag="ps_attn")
                for i, g in enumerate(gs4):
                    nc.tensor.transpose(
                        psV4[:, i, :65],
                        vpack[:, g, :, :].rearrange("p a b -> p (a b)"),
                        idb_sb[:65, :65],
                    )
                vt4_sb = attnsb.tile([128, 4, 65], bf16, tag="vt")
                nc.scalar.activation(
                    vt4_sb[:],
                    psV4[:, :, :65],
                    mybir.ActivationFunctionType.Copy,
                )
                psAV4 = psattn.tile([128, 4, 65], f32, tag="ps_attn")
                for i in range(4):
                    nc.tensor.matmul(
                        psAV4[:, i, :],
                        expS4[:, ds(i * 128, 128)],
                        vt4_sb[:, i, :],
                        start=True,
                        stop=True,
                    )
                rec4 = attnsb.tile([128, 4], f32, tag="rec")
                nc.vector.reciprocal(rec4[:], psAV4[:, :, 64])
                onorm4 = attnsb.tile([128, 4, 64], bf16, tag="onorm")
                nc.vector.tensor_tensor(
                    onorm4[:],
                    psAV4[:, :, 0:64],
                    rec4[:, :, None].to_broadcast((128, 4, 64)),
                    mybir.AluOpType.mult,
                )
                psN4 = psattn.tile([128, 2, 128], bf16, tag="ps_attn")
                for p in range(2):
                    nc.tensor.transpose(
                        psN4[:, p, :],
                        onorm4[:, 2 * p : 2 * p + 2, :].rearrange(
                            "p a b -> p (a b)"
                        ),
                        idb_sb[:],
                    )
                cp_alt = 0
                for p in range(2):
                    for i in range(2):
                        g = gs4[2 * p + i]
                        for par in range(2):
                            src = psN4[
                                i * 64 : i * 64 + 64, p, ds(par * 64, 64)
                            ].rearrange("p (a b) -> p a b", a=8)
                            dst = outtok[
                                par * 64 : par * 64 + 64, :, ds(g * 8, 8)
                            ]
                            if cp_alt % 2 == 0:
                                nc.vector.tensor_copy(out=dst, in_=src)
                            else:
                                nc.scalar.copy(out=dst, in_=src)
                            cp_alt += 1

            def emit_proj(it, outtok, jm):
                t0 = it * T
                for nh in range(2):
                    psO = psbig.tile([128, 512], f32, tag="ps_big")
                    for k2 in range(8):
                        nc.tensor.matmul(
                            psO,
                            outtok[:, k2, ds(jm * 128, 128)],
                            wout_sb[:, k2, ds(nh * 512, 512)],
                            start=(k2 == 0),
                            stop=False,
                        )
                    nc.tensor.matmul(
                        psO,
                        ones_sb[:, :],
                        bout_sb[:, ds(nh * 512, 512)],
                        start=False,
                        stop=True,
                    )
                    outf = outfp.tile([128, 512], f32, tag="outf")
                    nc.scalar.activation(
                        outf[:], psO, mybir.ActivationFunctionType.Copy
                    )
                    nc.sync.dma_start(
                        out_d[ds(t0 + jm * 128, 128), ds(nh * 512, 512)],
                        outf[:],
                    )

            # ---- software-pipelined schedule ----
            # prologue: QKV for iter 0
            xt_sb = xtp.tile([128, 8, T], bf16)
            for e in range(8):
                emit_xt(0, xt_sb, e)
            for j in range(F3 // 128):
                emit_qkv_ftile(0, xt_sb, j)

            for it in range(niter):
                nxt = it + 1
                outtok = outtokp.tile([128, 8, T], bf16, tag="outtok")
                if nxt < niter:
                    xt_nxt = xtp.tile([128, 8, T], bf16)
                for q4 in range(NQ):
                    emit_attn_quad(it, outtok, q4)
                    if nxt < niter:
                        if q4 < 2:
                            for e in range(4 * q4, 4 * q4 + 4):
                                emit_xt(nxt, xt_nxt, e)
                        else:
                            for j in range(4 * (q4 - 2), 4 * (q4 - 2) + 4):
                                emit_qkv_ftile(nxt, xt_nxt, j)
                    if q4 == NQ // 2 - 1:
                        emit_proj(it, outtok, 0)
                emit_proj(it, outtok, 1)
    nc.finalize()
    return nc


_cache = {}


def _get_nc(toks_per_core=TOKS):
    if toks_per_core not in _cache:
        _cache[toks_per_core] = build(toks_per_core)
    return _cache[toks_per_core]


def prep_inputs(x, w_qkv, b_qkv, w_out, b_out, toks_per_core=TOKS, n_cores=N_CORES):
    """Shard tokens over cores; replicate (host-preprocessed) weights."""
    xf = np.ascontiguousarray(np.asarray(x, dtype=np.float32).astype(ml_dtypes.bfloat16)).reshape(-1, E)
    wq = np.ascontiguousarray(np.asarray(w_qkv, dtype=np.float32).astype(ml_dtypes.bfloat16))
    bq = np.ascontiguousarray(
        np.asarray(b_qkv, dtype=np.float32).reshape(F3 // 128, 128).T
    )
    wo = np.ascontiguousarray(np.asarray(w_out).astype(ml_dtypes.bfloat16))
    bo = np.ascontiguousarray(np.asarray(b_out, dtype=np.float32).astype(ml_dtypes.bfloat16).reshape(1, E))
    in_maps = []
    for c in range(n_cores):
        in_maps.append(
            {
                "x": np.ascontiguousarray(
                    xf[c * toks_per_core : (c + 1) * toks_per_core]
                ),
                "w_qkv": wq,
                "b_qkv": bq,
                "w_out": wo,
                "b_out": bo,
            }
        )
    return in_maps


def run(x, w_qkv, b_qkv, w_out, b_out, toks_per_core=TOKS, n_cores=N_CORES, **kw):
    from concourse import bass_utils

    nc = _get_nc(toks_per_core)
    in_maps = prep_inputs(
        x, w_qkv, b_qkv, w_out, b_out, toks_per_core, n_cores
    )
    res = bass_utils.run_bass_kernel_spmd(
        nc, in_maps, core_ids=list(range(n_cores)), **kw
    )
    out = np.concatenate([r["out"] for r in res.results], axis=0)
    return out, res


def kernel(x, w_qkv, b_qkv, w_out, b_out):
    out, _ = run(x, w_qkv, b_qkv, w_out, b_out)
    return out.reshape(x.shape[0], x.shape[1], E)


# revision 28
# speedup vs baseline: 1.0731x; 1.0088x over previous
"""Bass/Tile TRN2 kernel for per-token multi-head attention over heads.

Reference computation (per token t):
  qkv = x @ w_qkv + b_qkv                  # [t, 3072]
  q/k/v[h, d] = qkv[h*192 + {0,64,128} + d]
  scores[h, g] = q[h] . k[g] / 8
  attn = softmax(scores, axis=g)
  out[h, d] = sum_g attn[h, g] v[g, d]
  y = out.reshape(1024) @ w_out + b_out

Sharding: tokens (B*S = 32768) split evenly over 8 cores; weights replicated.

Layout notes (v2):
  - qkv computed transposed ([f x t]) so per-head 64-row slabs are clean
    partition ranges; f32r matmuls (full PE rate at N>=256, ~1e-4 rounding).
  - per-head q/k/v repacked h-major into [d, head, t] bf16 tiles with
    contiguous copies; block-diag 128x128 matmuls give 8 tokens' 16x16
    head-attention at once (2 groups share each psum tile). Mask selects
    t==t' pairs; exp is unnormalized, softmax denominator comes from a
    ones-column in the AV matmul.
  - x transposed via DMA-transpose (2 x 64-partition transfers per e-chunk).
"""

import numpy as np
import ml_dtypes

H, DH = 16, 64
E = 1024
F3 = 3072
B, S = 4, 8192
N_CORES = 8
TOKS = (B * S) // N_CORES  # 4096 tokens per core
T = 256                    # tokens per unrolled iteration
NG = T // 8                # 8-token groups per iteration

NEG = -1.0e9


def _consts():
    # scoresT rows a=(slot_k, t), cols b=(slot_q, t'); valid iff t==t'
    a = np.arange(128)
    mask = np.where((a[:, None] % 8) == (a[None, :] % 8), 0.0, NEG).astype(
        np.float32
    )
    mask2 = np.concatenate([mask, mask], axis=1)  # [128, 256] for group pairs
    ident = np.eye(128, dtype=np.float32)
    return mask2, ident


def build(toks_per_core=TOKS):
    from concourse.bacc import Bacc
    import concourse.mybir as mybir
    from concourse.tile import TileContext
    from concourse.bass import ds

    f32 = mybir.dt.float32
    bf16 = mybir.dt.bfloat16
    niter = toks_per_core // T
    NQ = NG // 4  # quads per iteration

    nc = Bacc("TRN2")
    x_d = nc.dram_tensor("x", [toks_per_core, E], bf16, kind="ExternalInput")
    wqkv_d = nc.dram_tensor("w_qkv", [E, F3], bf16, kind="ExternalInput")
    bqkv_d = nc.dram_tensor("b_qkv", [128, F3 // 128], f32, kind="ExternalInput")
    wout_d = nc.dram_tensor("w_out", [E, E], bf16, kind="ExternalInput")
    bout_d = nc.dram_tensor("b_out", [1, E], bf16, kind="ExternalInput")
    out_d = nc.dram_tensor("out", [toks_per_core, E], f32, kind="ExternalOutput")

    _, ident_np = _consts()
    # mask fused into the scores matmul: extra contraction rows 64..71.
    # km[r, thi, s, tlo] = -sqrt(1e9) if tlo == r else 0
    # qm[r, s, t]        = +sqrt(1e9) if t % 8 != r else 0
    # => sum_r km.qm = -1e9 on t != t' pairs, exactly 0 on valid pairs
    SQ = np.float32(31623.0)
    tlo = np.arange(8)
    km_np = np.zeros((8, T // 8, H, 8), np.float32)
    km_np[tlo, :, :, tlo] = -SQ
    t_arr = np.arange(T)
    qm_np = np.where(
        (t_arr[None, None, :] % 8) != tlo[:, None, None], SQ, 0.0
    ).astype(np.float32)
    qm_np = np.broadcast_to(qm_np, (8, H, T)).copy()
    km_c = nc.inline_tensor(
        km_np.reshape(8, -1).astype(ml_dtypes.bfloat16), name="km_c"
    )
    qm_c = nc.inline_tensor(
        qm_np.reshape(8, -1).astype(ml_dtypes.bfloat16), name="qm_c"
    )
    identb_c = nc.inline_tensor(ident_np.astype(ml_dtypes.bfloat16), name="identb_c")
    ones_c = nc.inline_tensor(np.ones((1, 128), ml_dtypes.bfloat16), name="ones_c")

    # parity-major head slot: even heads 0-7, odd heads 8-15
    def hslot(h):
        return (h % 2) * 8 + h // 2

    with TileContext(nc) as tc:
        with (
            tc.tile_pool(name="persist", bufs=1) as pp,
            tc.tile_pool(name="xp", bufs=2) as xp,
            tc.tile_pool(name="xtp", bufs=2) as xtp,
            tc.tile_pool(name="attnsb", bufs=6) as attnsb,
            tc.tile_pool(name="outtokp", bufs=2) as outtokp,
            tc.tile_pool(name="outfp", bufs=3) as outfp,
            tc.tile_pool(name="psbig", bufs=4, space="PSUM") as psbig,
            tc.tile_pool(name="psattn", bufs=4, space="PSUM") as psattn,
        ):
            # ---- resident weights / constants ----
            w_sb = pp.tile([128, 8, F3], bf16)
            nc.sync.dma_start(w_sb, wqkv_d.rearrange("(ko kp) f -> kp ko f", kp=128))
            wout_sb = pp.tile([128, 8, E], bf16)
            nc.sync.dma_start(wout_sb, wout_d.rearrange("(ko kp) f -> kp ko f", kp=128))
            bqkv_sb = pp.tile([128, F3 // 128], f32)
            nc.sync.dma_start(bqkv_sb, bqkv_d[:])
            bout_sb = pp.tile([1, E], bf16)
            nc.sync.dma_start(bout_sb, bout_d[:])
            idb_sb = pp.tile([128, 128], bf16)
            nc.sync.dma_start(idb_sb, identb_c[:])
            ones_sb = pp.tile([1, 128], bf16)
            nc.sync.dma_start(ones_sb, ones_c[:])

            # double-buffered packs; mask rows are persistent constants
            qpacks, kpacks, vpacks = [], [], []
            for sidx in range(2):
                qpack = pp.tile([72, H, T], bf16, name=f"qpack{sidx}")
                kpack = pp.tile([72, T // 8, H, 8], bf16, name=f"kpack{sidx}")
                vpack = pp.tile([65, T // 8, H, 8], bf16, name=f"vpack{sidx}")
                nc.gpsimd.memset(vpack[64:65, :, :, :], 1.0)
                nc.sync.dma_start(
                    qpack[64:72, :, :].rearrange("p a b -> p (a b)"), qm_c[:]
                )
                nc.sync.dma_start(
                    kpack[64:72, :, :, :].rearrange("p a b c -> p (a b c)"),
                    km_c[:],
                )
                qpacks.append(qpack)
                kpacks.append(kpack)
                vpacks.append(vpack)

            def emit_xt(it, xt_sb, e):
                t0 = it * T
                nc.sync.dma_start_transpose(
                    xt_sb[:, e, :],
                    x_d[ds(t0, T), ds(e * 128, 128)],
                )

            drain_alt = [0]

            def emit_qkv_ftile(it, xt_sb, j):
                # f-tile j -> psum -> two 64-row slab drains into packs
                sidx = it % 2
                psq_full = psbig.tile([128, 512], f32, tag="ps_big")
                psq = psq_full[:, :T]
                for e in range(8):
                    nc.tensor.matmul(
                        psq,
                        w_sb[:, e, ds(j * 128, 128)],
                        xt_sb[:, e, :],
                        start=(e == 0),
                        stop=(e == 7),
                    )
                for half in range(2):
                    frow = j * 128 + half * 64
                    h, rem = divmod(frow, 192)
                    which = rem // 64
                    sl = hslot(h)
                    if which == 0:
                        dst = qpacks[sidx][:64, sl, :]
                    elif which == 1:
                        dst = kpacks[sidx][:64, :, sl, :]
                    else:
                        dst = vpacks[sidx][:64, :, sl, :]
                    src = psq[half * 64 : half * 64 + 64, :]
                    bias_ap = bqkv_sb[half * 64 : half * 64 + 64, j : j + 1]
                    if which == 0:
                        nc.scalar.activation(
                            dst,
                            src,
                            mybir.ActivationFunctionType.Identity,
                            bias=bias_ap,
                            scale=1.0,
                        )
                    else:
                        nc.vector.scalar_tensor_tensor(
                            out=dst,
                            in0=src.rearrange("p (a b) -> p a b", b=8),
                            scalar=1.0,
                            in1=bias_ap[:, :, None].to_broadcast(
                                (64, T // 8, 8)
                            ),
                            op0=mybir.AluOpType.mult,
                            op1=mybir.AluOpType.add,
                        )

            def emit_attn_quad(it, outtok, q4):
                sidx = it % 2
                qpack, kpack, vpack = qpacks[sidx], kpacks[sidx], vpacks[sidx]
                gs4 = [4 * q4 + i for i in range(4)]
                psS4 = psattn.tile([128, 4, 128], f32, tag="ps_attn")
                for i, g in enumerate(gs4):
                    nc.tensor.matmul(
                        psS4[:, i, :],
                        kpack[:, g, :, :].rearrange("p a b -> p (a b)"),
                        qpack[:, :, ds(g * 8, 8)],
                        start=True,
                        stop=True,
                    )
                expS4 = attnsb.tile([128, 512], bf16, tag="expS")
                nc.scalar.activation(
                    expS4[:],
                    psS4.rearrange("p a b -> p (a b)"),
                    mybir.ActivationFunctionType.Exp,
                    bias=0.0,
                    scale=0.125,
                )
                psV4 = psattn.tile([128, 4, 66], bf16, tag="ps_attn")
                for i, g in enumerate(gs4):
                    nc.tensor.transpose(
                        psV4[:, i, :65],
                        vpack[:, g, :, :].rearrange("p a b -> p (a b)"),
                        idb_sb[:65, :65],
                    )
                vt4_sb = attnsb.tile([128, 4, 65], bf16, tag="vt")
                nc.scalar.activation(
                    vt4_sb[:],
                    psV4[:, :, :65],
                    mybir.ActivationFunctionType.Copy,
                )
                psAV4 = psattn.tile([128, 4, 65], f32, tag="ps_attn")
                for i in range(4):
                    nc.tensor.matmul(
                        psAV4[:, i, :],
                        expS4[:, ds(i * 128, 128)],
                        vt4_sb[:, i, :],
                        start=True,
                        stop=True,
                    )
                rec4 = attnsb.tile([128, 4], f32, tag="rec")
                nc.vector.reciprocal(rec4[:], psAV4[:, :, 64])
                onorm4 = attnsb.tile([128, 4, 64], bf16, tag="onorm")
                nc.vector.tensor_tensor(
                    onorm4[:],
                    psAV4[:, :, 0:64],
                    rec4[:, :, None].to_broadcast((128, 4, 64)),
                    mybir.AluOpType.mult,
                )
                psN4 = psattn.tile([128, 2, 128], bf16, tag="ps_attn")
                for p in range(2):
                    nc.tensor.transpose(
                        psN4[:, p, :],
                        onorm4[:, 2 * p : 2 * p + 2, :].rearrange(
                            "p a b -> p (a b)"
                        ),
                        idb_sb[:],
                    )
                cp_alt = 0
                for p in range(2):
                    for i in range(2):
                        g = gs4[2 * p + i]
                        for par in range(2):
                            src = psN4[
                                i * 64 : i * 64 + 64, p, ds(par * 64, 64)
                            ].rearrange("p (a b) -> p a b", a=8)
                            dst = outtok[
                                par * 64 : par * 64 + 64, :, ds(g * 8, 8)
                            ]
                            if cp_alt % 2 == 0:
                                nc.vector.tensor_copy(out=dst, in_=src)
                            else:
                                nc.scalar.copy(out=dst, in_=src)
                            cp_alt += 1

            def emit_proj(it, outtok, jm):
                t0 = it * T
                for nh in range(2):
                    psO = psbig.tile([128, 512], f32, tag="ps_big")
                    for k2 in range(8):
                        nc.tensor.matmul(
                            psO,
                            outtok[:, k2, ds(jm * 128, 128)],
                            wout_sb[:, k2, ds(nh * 512, 512)],
                            start=(k2 == 0),
                            stop=False,
                        )
                    nc.tensor.matmul(
                        psO,
                        ones_sb[:, :],
                        bout_sb[:, ds(nh * 512, 512)],
                        start=False,
                        stop=True,
                    )
                    outf = outfp.tile([128, 512], f32, tag="outf")
                    nc.scalar.activation(
                        outf[:], psO, mybir.ActivationFunctionType.Copy
                    )
                    nc.sync.dma_start(
                        out_d[ds(t0 + jm * 128, 128), ds(nh * 512, 512)],
                        outf[:],
                    )

            # ---- software-pipelined schedule ----
            # prologue: QKV for iter 0
            xt_sb = xtp.tile([128, 8, T], bf16)
            for e in range(8):
                emit_xt(0, xt_sb, e)
            for j in range(F3 // 128):
                emit_qkv_ftile(0, xt_sb, j)

            for it in range(niter):
                nxt = it + 1
                outtok = outtokp.tile([128, 8, T], bf16, tag="outtok")
                if nxt < niter:
                    xt_nxt = xtp.tile([128, 8, T], bf16)
                for q4 in range(NQ):
                    emit_attn_quad(it, outtok, q4)
                    if nxt < niter:
                        if q4 < 2:
                            for e in range(4 * q4, 4 * q4 + 4):
                                emit_xt(nxt, xt_nxt, e)
                        else:
                            for j in range(4 * (q4 - 2), 4 * (q4 - 2) + 4):
                                emit_qkv_ftile(nxt, xt_nxt, j)
                    if q4 == NQ // 2 - 1:
                        emit_proj(it, outtok, 0)
                emit_proj(it, outtok, 1)
    nc.finalize()
    return nc


_cache = {}


def _get_nc(toks_per_core=TOKS):
    if toks_per_core not in _cache:
        _cache[toks_per_core] = build(toks_per_core)
    return _cache[toks_per_core]


def prep_inputs(x, w_qkv, b_qkv, w_out, b_out, toks_per_core=TOKS, n_cores=N_CORES):
    """Shard tokens over cores; replicate (host-preprocessed) weights."""
    xf = np.ascontiguousarray(np.asarray(x, dtype=np.float32).astype(ml_dtypes.bfloat16)).reshape(-1, E)
    wq = np.ascontiguousarray(np.asarray(w_qkv, dtype=np.float32).astype(ml_dtypes.bfloat16))
    bq = np.ascontiguousarray(
        np.asarray(b_qkv, dtype=np.float32).reshape(F3 // 128, 128).T
    )
    wo = np.ascontiguousarray(np.asarray(w_out).astype(ml_dtypes.bfloat16))
    bo = np.ascontiguousarray(np.asarray(b_out, dtype=np.float32).astype(ml_dtypes.bfloat16).reshape(1, E))
    in_maps = []
    for c in range(n_cores):
        in_maps.append(
            {
                "x": np.ascontiguousarray(
                    xf[c * toks_per_core : (c + 1) * toks_per_core]
                ),
                "w_qkv": wq,
                "b_qkv": bq,
                "w_out": wo,
                "b_out": bo,
            }
        )
    return in_maps


def run(x, w_qkv, b_qkv, w_out, b_out, toks_per_core=TOKS, n_cores=N_CORES, **kw):
    from concourse import bass_utils

    nc = _get_nc(toks_per_core)
    in_maps = prep_inputs(
        x, w_qkv, b_qkv, w_out, b_out, toks_per_core, n_cores
    )
    res = bass_utils.run_bass_kernel_spmd(
        nc, in_maps, core_ids=list(range(n_cores)), **kw
    )
    out = np.concatenate([r["out"] for r in res.results], axis=0)
    return out, res


def kernel(x, w_qkv, b_qkv, w_out, b_out):
    out, _ = run(x, w_qkv, b_qkv, w_out, b_out)
    return out.reshape(x.shape[0], x.shape[1], E)
